# revision 2
# baseline (speedup 1.0000x reference)
"""Batched Chamfer loss on 8 Trainium2 cores.

Strategy (data-parallel over batch, 2 batches/core):
  d2[n,m] = ||s_n||^2 + ||t_m||^2 - 2 s_n.t_m is produced directly by one
  K=18 bf16 matmul per [128,512] tile using a split-precision packing:
    - 12 rows: hi/lo bf16 split of (-2*s_d) x (t_d)  (exact products, fp32 accum)
    - 3 rows:  ||s||^2 as 3-term bf16 split (x ones)
    - 3 rows:  ones (x ||t||^2 as 3-term bf16 split)
  ScalarE evacuates PSUM -> SBUF as fp16 *negated* (scale=-1), so both
  chamfer directions become max-reductions: VectorE does a fused row-max
  (tensor_scalar max-accumulate) and a running col-max (tensor_tensor max,
  fp16 2x mode). GPSIMD folds the col-max across partitions
  (partition_all_reduce(max)). Host negates and does the tiny final means.
"""
import numpy as np
import ml_dtypes

B, N, M = 16, 4096, 4096
NCORES = 8
BPC = B // NCORES          # batches per core
K = 18                     # packed contraction rows
NT = N // 128              # 32 n-tiles
NH = 2                     # m halves
HW = M // NH               # 2048 columns per half
BF16 = ml_dtypes.bfloat16

_cache = {}


def _split2(x):
    """fp32 array -> (hi, lo) bf16 so hi+lo ~ x to ~2^-17 rel."""
    hi = x.astype(BF16)
    lo = (x - hi.astype(np.float32)).astype(BF16)
    return hi, lo


def _split3(x):
    a = x.astype(BF16)
    r = x - a.astype(np.float32)
    b = r.astype(BF16)
    c = (r - b.astype(np.float32)).astype(BF16)
    return a, b, c


def _pack_batch(s, t):
    """s: [N,3] f32, t: [M,3] f32 -> a_pack [K,N] bf16, b_pack [K,M] bf16."""
    a = np.zeros((K, N), dtype=BF16)
    bp = np.zeros((K, M), dtype=BF16)
    for d in range(3):
        xh, xl = _split2(-2.0 * s[:, d])
        th, tl = _split2(t[:, d])
        r = 4 * d
        a[r + 0] = xh
        a[r + 1] = xh
        a[r + 2] = xl
        a[r + 3] = xl
        bp[r + 0] = th
        bp[r + 1] = tl
        bp[r + 2] = th
        bp[r + 3] = tl
    s2 = (s.astype(np.float64) ** 2).sum(-1).astype(np.float32)
    t2 = (t.astype(np.float64) ** 2).sum(-1).astype(np.float32)
    s2a, s2b, s2c = _split3(s2)
    a[12], a[13], a[14] = s2a, s2b, s2c
    bp[12] = bp[13] = bp[14] = np.ones(M, dtype=BF16)
    a[15] = a[16] = a[17] = np.ones(N, dtype=BF16)
    t2a, t2b, t2c = _split3(t2)
    bp[15], bp[16], bp[17] = t2a, t2b, t2c
    return a, bp


def _build():
    import concourse.bacc as bacc
    import concourse.mybir as mybir
    import concourse.tile as tile
    import concourse.bass_isa as bass_isa

    f32 = mybir.dt.float32
    f16 = mybir.dt.float16
    bf16 = mybir.dt.bfloat16
    MAX = mybir.AluOpType.max
    BYP = mybir.AluOpType.bypass

    nc = bacc.Bacc("TRN2", target_bir_lowering=False, debug=False)
    apack = nc.dram_tensor("apack", [BPC, K, N], bf16, kind="ExternalInput")
    bpack = nc.dram_tensor("bpack", [BPC, K, M], bf16, kind="ExternalInput")
    # outputs hold NEGATED mins
    o_rm = nc.dram_tensor("rowmins", [BPC, 128, NT], f32, kind="ExternalOutput")
    o_cm = nc.dram_tensor("colmin", [BPC, M], f32, kind="ExternalOutput")

    with tile.TileContext(nc) as tc:
        with (
            tc.tile_pool(name="w", bufs=2) as wpool,
            tc.tile_pool(name="ps", bufs=2, space="PSUM") as pspool,
            tc.tile_pool(name="span", bufs=3) as sppool,
            tc.tile_pool(name="cm", bufs=2) as cmpool,
            tc.tile_pool(name="rm", bufs=2) as rmpool,
            tc.tile_pool(name="rmb", bufs=3) as rmbpool,
            tc.tile_pool(name="fold", bufs=2) as fpool,
        ):
            for b in range(BPC):
                a_sb = wpool.tile([K, N], bf16, tag="a")
                bt_sb = wpool.tile([K, M], bf16, tag="bt")
                nc.sync.dma_start(a_sb[:], apack[b])
                nc.sync.dma_start(bt_sb[:], bpack[b])

                colmax = cmpool.tile([128, M], f16)
                nc.vector.memset(colmax[:], float("-inf"))
                rowm = rmpool.tile([128, NT], f32)

                for t in range(NT):
                    rmbuf = rmbpool.tile([128, NH], f32)
                    for h in range(NH):
                        ps = pspool.tile([128, HW], f32)
                        for i in range(HW // 512):
                            nc.tensor.matmul(
                                ps[:, i * 512:(i + 1) * 512],
                                a_sb[:, t * 128:(t + 1) * 128],
                                bt_sb[:, h * HW + i * 512: h * HW + (i + 1) * 512],
                                start=True,
                                stop=True,
                            )
                        span = sppool.tile([128, HW], f16)
                        # evacuate + negate: span = -d2
                        nc.scalar.mul(span[:], ps[:], -1.0)
                        # fused row-max: accum_out = max over free dim
                        nc.vector.tensor_scalar(
                            out=span[:],
                            in0=span[:],
                            scalar1=0.0,
                            scalar2=None,
                            op0=BYP,
                            op1=MAX,
                            accum_out=rmbuf[:, h:h + 1],
                        )
                        # running col-max
                        nc.vector.tensor_tensor(
                            out=colmax[:, h * HW:(h + 1) * HW],
                            in0=span[:],
                            in1=colmax[:, h * HW:(h + 1) * HW],
                            op=MAX,
                        )
                    nc.vector.tensor_reduce(
                        rowm[:, t:t + 1], rmbuf[:], axis=mybir.AxisListType.X, op=MAX
                    )

                # fold col-max across partitions on gpsimd
                cfold = fpool.tile([128, M], f16)
                nc.gpsimd.partition_all_reduce(
                    cfold[:], colmax[:], 128, bass_isa.ReduceOp.max
                )
                cm_f32 = fpool.tile([1, M], f32, tag="cmf32")
                nc.vector.tensor_copy(cm_f32[:], cfold[0:1, :])
                nc.sync.dma_start(o_rm[b], rowm[:])
                nc.sync.dma_start(o_cm[b], cm_f32[:])
    nc.compile()
    return nc


def _get_module():
    if "nc" not in _cache:
        _cache["nc"] = _build()
    return _cache["nc"]


def _make_in_maps(src_points, tgt_points):
    in_maps = []
    for c in range(NCORES):
        ap = np.empty((BPC, K, N), dtype=BF16)
        bp = np.empty((BPC, K, M), dtype=BF16)
        for j in range(BPC):
            b = c * BPC + j
            ap[j], bp[j] = _pack_batch(src_points[b], tgt_points[b])
        in_maps.append({"apack": ap, "bpack": bp})
    return in_maps


def _host_reduce(results, weights):
    total = 0.0
    for c in range(NCORES):
        rm = results[c]["rowmins"].astype(np.float64)  # [BPC,128,NT], -rowmin
        cm = results[c]["colmin"].astype(np.float64)   # [BPC,M], -colmin
        for j in range(BPC):
            b = c * BPC + j
            d1 = np.maximum(-rm[j], 0.0).mean()
            d2 = np.maximum(-cm[j], 0.0).mean()
            total += float(weights[b]) * (d1 + d2)
    return np.float32(total / B)


def kernel(src_points, tgt_points, weights):
    from concourse.bass_utils import run_bass_kernel_spmd

    src_points = np.asarray(src_points, dtype=np.float32)
    tgt_points = np.asarray(tgt_points, dtype=np.float32)
    weights = np.asarray(weights, dtype=np.float32)

    nc = _get_module()
    in_maps = _make_in_maps(src_points, tgt_points)
    res = run_bass_kernel_spmd(nc, in_maps, list(range(NCORES)))
    return _host_reduce(res.results, weights)


# revision 4
# speedup vs baseline: 9.5626x; 9.5626x over previous
"""Batched Chamfer loss on 8 Trainium2 cores.

Strategy (data-parallel over batch, 2 batches/core):
  d2[n,m] = ||s_n||^2 + ||t_m||^2 - 2 s_n.t_m is produced directly by one
  K=18 bf16 matmul per [128,512] tile using a split-precision packing:
    - 12 rows: hi/lo bf16 split of (-2*s_d) x (t_d)  (exact products, fp32 accum)
    - 3 rows:  ||s||^2 as 3-term bf16 split (x ones)
    - 3 rows:  ones (x ||t||^2 as 3-term bf16 split)
  ScalarE evacuates PSUM -> SBUF as fp16 *negated* (scale=-1), so both
  chamfer directions become max-reductions: VectorE does a fused row-max
  (tensor_scalar max-accumulate) and a running col-max (tensor_tensor max,
  fp16 2x mode). GPSIMD folds the col-max across partitions
  (partition_all_reduce(max)). Host negates and does the tiny final means.
"""
import numpy as np
import ml_dtypes

B, N, M = 16, 4096, 4096
NCORES = 8
BPC = B // NCORES          # batches per core
K = 18                     # packed contraction rows
NT = N // 128              # 32 n-tiles
NH = 2                     # m halves
HW = M // NH               # 2048 columns per half
BF16 = ml_dtypes.bfloat16

_cache = {}


def _split2(x):
    """fp32 array -> (hi, lo) bf16 so hi+lo ~ x to ~2^-17 rel."""
    hi = x.astype(BF16)
    lo = (x - hi.astype(np.float32)).astype(BF16)
    return hi, lo


def _split3(x):
    a = x.astype(BF16)
    r = x - a.astype(np.float32)
    b = r.astype(BF16)
    c = (r - b.astype(np.float32)).astype(BF16)
    return a, b, c


def _pack_batch(s, t):
    """s: [N,3] f32, t: [M,3] f32 -> a_pack [K,N] bf16, b_pack [K,M] bf16."""
    a = np.zeros((K, N), dtype=BF16)
    bp = np.zeros((K, M), dtype=BF16)
    for d in range(3):
        xh, xl = _split2(-2.0 * s[:, d])
        th, tl = _split2(t[:, d])
        r = 4 * d
        a[r + 0] = xh
        a[r + 1] = xh
        a[r + 2] = xl
        a[r + 3] = xl
        bp[r + 0] = th
        bp[r + 1] = tl
        bp[r + 2] = th
        bp[r + 3] = tl
    s2 = (s.astype(np.float64) ** 2).sum(-1).astype(np.float32)
    t2 = (t.astype(np.float64) ** 2).sum(-1).astype(np.float32)
    s2a, s2b, s2c = _split3(s2)
    a[12], a[13], a[14] = s2a, s2b, s2c
    bp[12] = bp[13] = bp[14] = np.ones(M, dtype=BF16)
    a[15] = a[16] = a[17] = np.ones(N, dtype=BF16)
    t2a, t2b, t2c = _split3(t2)
    bp[15], bp[16], bp[17] = t2a, t2b, t2c
    return a, bp


def _build(reps=1):
    import concourse.bacc as bacc
    import concourse.mybir as mybir
    import concourse.tile as tile
    import concourse.bass_isa as bass_isa

    f32 = mybir.dt.float32
    f16 = mybir.dt.float16
    bf16 = mybir.dt.bfloat16
    MAX = mybir.AluOpType.max
    BYP = mybir.AluOpType.bypass

    nc = bacc.Bacc("TRN2", target_bir_lowering=False, debug=False)
    apack = nc.dram_tensor("apack", [BPC, K, N], bf16, kind="ExternalInput")
    bpack = nc.dram_tensor("bpack", [BPC, K, M], bf16, kind="ExternalInput")
    # outputs hold NEGATED mins
    o_rm = nc.dram_tensor("rowmins", [BPC, 128, NT], f32, kind="ExternalOutput")
    o_cm = nc.dram_tensor("colmin", [BPC, M], f32, kind="ExternalOutput")

    with tile.TileContext(nc) as tc:
        with (
            tc.tile_pool(name="w", bufs=2) as wpool,
            tc.tile_pool(name="ps", bufs=2, space="PSUM") as pspool,
            tc.tile_pool(name="span", bufs=3) as sppool,
            tc.tile_pool(name="cm", bufs=2) as cmpool,
            tc.tile_pool(name="rm", bufs=2) as rmpool,
            tc.tile_pool(name="rmb", bufs=3) as rmbpool,
            tc.tile_pool(name="fold", bufs=2) as fpool,
        ):
            for b_rep in range(BPC * reps):
                b = b_rep % BPC
                a_sb = wpool.tile([K, N], bf16, tag="a")
                bt_sb = wpool.tile([K, M], bf16, tag="bt")
                nc.sync.dma_start(a_sb[:], apack[b])
                nc.sync.dma_start(bt_sb[:], bpack[b])

                colmax = cmpool.tile([128, M], f16)
                nc.vector.memset(colmax[:], float("-inf"))
                rowm = rmpool.tile([128, NT], f32)

                for t in range(NT):
                    rmbuf = rmbpool.tile([128, NH], f32)
                    for h in range(NH):
                        ps = pspool.tile([128, HW], f32)
                        for i in range(HW // 512):
                            nc.tensor.matmul(
                                ps[:, i * 512:(i + 1) * 512],
                                a_sb[:, t * 128:(t + 1) * 128],
                                bt_sb[:, h * HW + i * 512: h * HW + (i + 1) * 512],
                                start=True,
                                stop=True,
                            )
                        span = sppool.tile([128, HW], f16)
                        # evacuate + negate: span = -d2
                        nc.scalar.mul(span[:], ps[:], -1.0)
                        # fused row-max: accum_out = max over free dim
                        nc.vector.tensor_scalar(
                            out=span[:],
                            in0=span[:],
                            scalar1=0.0,
                            scalar2=None,
                            op0=BYP,
                            op1=MAX,
                            accum_out=rmbuf[:, h:h + 1],
                        )
                        # running col-max
                        nc.vector.tensor_tensor(
                            out=colmax[:, h * HW:(h + 1) * HW],
                            in0=span[:],
                            in1=colmax[:, h * HW:(h + 1) * HW],
                            op=MAX,
                        )
                    nc.vector.tensor_reduce(
                        rowm[:, t:t + 1], rmbuf[:], axis=mybir.AxisListType.X, op=MAX
                    )

                # fold col-max across partitions on gpsimd
                cfold = fpool.tile([128, M], f16)
                nc.gpsimd.partition_all_reduce(
                    cfold[:], colmax[:], 128, bass_isa.ReduceOp.max
                )
                cm_f32 = fpool.tile([1, M], f32, tag="cmf32")
                nc.vector.tensor_copy(cm_f32[:], cfold[0:1, :])
                nc.sync.dma_start(o_rm[b], rowm[:])
                nc.sync.dma_start(o_cm[b], cm_f32[:])
    nc.compile()
    return nc


def _get_module():
    if "nc" not in _cache:
        _cache["nc"] = _build()
    return _cache["nc"]


def _make_in_maps(src_points, tgt_points):
    in_maps = []
    for c in range(NCORES):
        ap = np.empty((BPC, K, N), dtype=BF16)
        bp = np.empty((BPC, K, M), dtype=BF16)
        for j in range(BPC):
            b = c * BPC + j
            ap[j], bp[j] = _pack_batch(src_points[b], tgt_points[b])
        in_maps.append({"apack": ap, "bpack": bp})
    return in_maps


def _host_reduce(results, weights):
    total = 0.0
    for c in range(NCORES):
        rm = results[c]["rowmins"].astype(np.float64)  # [BPC,128,NT], -rowmin
        cm = results[c]["colmin"].astype(np.float64)   # [BPC,M], -colmin
        for j in range(BPC):
            b = c * BPC + j
            d1 = np.maximum(-rm[j], 0.0).mean()
            d2 = np.maximum(-cm[j], 0.0).mean()
            total += float(weights[b]) * (d1 + d2)
    return np.float32(total / B)


def kernel(src_points, tgt_points, weights):
    from concourse.bass_utils import run_bass_kernel_spmd

    src_points = np.asarray(src_points, dtype=np.float32)
    tgt_points = np.asarray(tgt_points, dtype=np.float32)
    weights = np.asarray(weights, dtype=np.float32)

    nc = _get_module()
    in_maps = _make_in_maps(src_points, tgt_points)
    res = run_bass_kernel_spmd(nc, in_maps, list(range(NCORES)))
    return _host_reduce(res.results, weights)


# revision 16
# speedup vs baseline: 10.0507x; 1.0510x over previous
"""Batched Chamfer loss on 8 Trainium2 cores.

Strategy (data-parallel over batch, 2 batches/core):
  d2[n,m] = ||s_n||^2 + ||t_m||^2 - 2 s_n.t_m is produced directly by one
  K=18 bf16 matmul per [128,512] tile using a split-precision packing:
    - 12 rows: hi/lo bf16 split of (-2*s_d) x (t_d)  (exact products, fp32 accum)
    - 3 rows:  ||s||^2 as 3-term bf16 split (x ones)
    - 3 rows:  ones (x ||t||^2 as 3-term bf16 split)
  ScalarE evacuates PSUM -> SBUF as fp16 *negated* (scale=-1), so both
  chamfer directions become max-reductions: VectorE does a fused row-max
  (tensor_scalar max-accumulate) and a running col-max (tensor_tensor max,
  fp16 2x mode). GPSIMD folds the col-max across partitions
  (partition_all_reduce(max)). Host negates and does the tiny final means.
"""
import numpy as np
import ml_dtypes

B, N, M = 16, 4096, 4096
NCORES = 8
BPC = B // NCORES          # batches per core
K = 18                     # packed contraction rows
NT = N // 128              # 32 n-tiles
NH = 2                     # m halves
HW = M // NH               # 2048 columns per half
BF16 = ml_dtypes.bfloat16

_cache = {}


def _split2(x):
    """fp32 array -> (hi, lo) bf16 so hi+lo ~ x to ~2^-17 rel."""
    hi = x.astype(BF16)
    lo = (x - hi.astype(np.float32)).astype(BF16)
    return hi, lo


def _split3(x):
    a = x.astype(BF16)
    r = x - a.astype(np.float32)
    b = r.astype(BF16)
    c = (r - b.astype(np.float32)).astype(BF16)
    return a, b, c


def _pack_batch(s, t):
    """s: [N,3] f32, t: [M,3] f32 -> a_pack [K,N] bf16, b_pack [K,M] bf16."""
    a = np.zeros((K, N), dtype=BF16)
    bp = np.zeros((K, M), dtype=BF16)
    for d in range(3):
        xh, xl = _split2(-2.0 * s[:, d])
        th, tl = _split2(t[:, d])
        r = 4 * d
        a[r + 0] = xh
        a[r + 1] = xh
        a[r + 2] = xl
        a[r + 3] = xl
        bp[r + 0] = th
        bp[r + 1] = tl
        bp[r + 2] = th
        bp[r + 3] = tl
    s2 = (s.astype(np.float64) ** 2).sum(-1).astype(np.float32)
    t2 = (t.astype(np.float64) ** 2).sum(-1).astype(np.float32)
    s2a, s2b, s2c = _split3(s2)
    a[12], a[13], a[14] = s2a, s2b, s2c
    bp[12] = bp[13] = bp[14] = np.ones(M, dtype=BF16)
    a[15] = a[16] = a[17] = np.ones(N, dtype=BF16)
    t2a, t2b, t2c = _split3(t2)
    bp[15], bp[16], bp[17] = t2a, t2b, t2c
    return a, bp


def _build(reps=1, drop_ts=False, drop_tt=False, drop_act=False, ts_sep_out=False,
           gnum=0, gden=1, ttr_rowmax=False, span_bufs=3):
    import concourse.bacc as bacc
    import concourse.mybir as mybir
    import concourse.tile as tile
    import concourse.bass_isa as bass_isa

    f32 = mybir.dt.float32
    f16 = mybir.dt.float16
    bf16 = mybir.dt.bfloat16
    MAX = mybir.AluOpType.max
    BYP = mybir.AluOpType.bypass

    nc = bacc.Bacc("TRN2", target_bir_lowering=False, debug=False)
    apack = nc.dram_tensor("apack", [BPC, K, N], bf16, kind="ExternalInput")
    bpack = nc.dram_tensor("bpack", [BPC, K, M], bf16, kind="ExternalInput")
    # outputs hold NEGATED mins
    o_rm = nc.dram_tensor("rowmins", [BPC, 128, NT], f32, kind="ExternalOutput")
    o_cm = nc.dram_tensor("colmin", [BPC, M], f32, kind="ExternalOutput")

    with tile.TileContext(nc) as tc:
        with (
            tc.tile_pool(name="w", bufs=2) as wpool,
            tc.tile_pool(name="ps", bufs=2, space="PSUM") as pspool,
            tc.tile_pool(name="span", bufs=span_bufs) as sppool,
            tc.tile_pool(name="cm", bufs=2) as cmpool,
            tc.tile_pool(name="rm", bufs=2) as rmpool,
            tc.tile_pool(name="rmb", bufs=3) as rmbpool,
            tc.tile_pool(name="fold", bufs=2) as fpool,
        ):
            for b_rep in range(BPC * reps):
                b = b_rep % BPC
                a_sb = wpool.tile([K, N], bf16, tag="a")
                bt_sb = wpool.tile([K, M], bf16, tag="bt")
                nc.sync.dma_start(a_sb[:], apack[b])
                nc.sync.dma_start(bt_sb[:], bpack[b])

                colmax = cmpool.tile([128, M], f16)
                nc.vector.memset(colmax[:], float("-inf"))
                rowm = rmpool.tile([128, NT], f32)
                if drop_ts:
                    nc.vector.memset(rowm[:], 0.0)
                neginf = None
                if ttr_rowmax:
                    neginf = rmpool.tile([128, 1], f32, tag="neginf")
                    nc.vector.memset(neginf[:], -3.0e38)

                for t in range(NT):
                    span = sppool.tile([128, M], f16)
                    for h in range(NH):
                        ps = pspool.tile([128, HW], f32)
                        for i in range(HW // 512):
                            nc.tensor.matmul(
                                ps[:, i * 512:(i + 1) * 512],
                                a_sb[:, t * 128:(t + 1) * 128],
                                bt_sb[:, h * HW + i * 512: h * HW + (i + 1) * 512],
                                start=True,
                                stop=True,
                            )
                        # evacuate + negate: span = -d2
                        if not drop_act:
                            nc.scalar.mul(span[:, h * HW:(h + 1) * HW], ps[:], -1.0)
                    # fused row-max over the whole n-tile row
                    if not drop_ts:
                        if ttr_rowmax:
                            hmax = rmbpool.tile([128, HW], f16, tag="hmax")
                            nc.vector.tensor_tensor_reduce(
                                out=hmax[:],
                                in0=span[:, 0:HW],
                                in1=span[:, HW:M],
                                scale=1.0,
                                scalar=neginf[:],
                                op0=MAX,
                                op1=MAX,
                                accum_out=rowm[:, t:t + 1],
                            )
                        else:
                            nc.vector.tensor_scalar(
                                out=span[:],
                                in0=span[:],
                                scalar1=0.0,
                                scalar2=None,
                                op0=BYP,
                                op1=MAX,
                                accum_out=rowm[:, t:t + 1],
                            )
                    # running col-max (optionally on gpsimd)
                    if not drop_tt:
                        eng = nc.gpsimd if (t * gnum) % gden < gnum else nc.vector
                        eng.tensor_tensor(
                            out=colmax[:], in0=span[:], in1=colmax[:], op=MAX
                        )

                # fold col-max across partitions on gpsimd
                cfold = fpool.tile([128, M], f16)
                nc.gpsimd.partition_all_reduce(
                    cfold[:], colmax[:], 128, bass_isa.ReduceOp.max
                )
                cm_f32 = fpool.tile([1, M], f32, tag="cmf32")
                nc.vector.tensor_copy(cm_f32[:], cfold[0:1, :])
                nc.sync.dma_start(o_rm[b], rowm[:])
                nc.sync.dma_start(o_cm[b], cm_f32[:])
    nc.compile()
    return nc


def _get_module():
    if "nc" not in _cache:
        _cache["nc"] = _build()
    return _cache["nc"]


def _make_in_maps(src_points, tgt_points):
    in_maps = []
    for c in range(NCORES):
        ap = np.empty((BPC, K, N), dtype=BF16)
        bp = np.empty((BPC, K, M), dtype=BF16)
        for j in range(BPC):
            b = c * BPC + j
            ap[j], bp[j] = _pack_batch(src_points[b], tgt_points[b])
        in_maps.append({"apack": ap, "bpack": bp})
    return in_maps


def _host_reduce(results, weights):
    total = 0.0
    for c in range(NCORES):
        rm = results[c]["rowmins"].astype(np.float64)  # [BPC,128,NT], -rowmin
        cm = results[c]["colmin"].astype(np.float64)   # [BPC,M], -colmin
        for j in range(BPC):
            b = c * BPC + j
            d1 = np.maximum(-rm[j], 0.0).mean()
            d2 = np.maximum(-cm[j], 0.0).mean()
            total += float(weights[b]) * (d1 + d2)
    return np.float32(total / B)


def kernel(src_points, tgt_points, weights):
    from concourse.bass_utils import run_bass_kernel_spmd

    src_points = np.asarray(src_points, dtype=np.float32)
    tgt_points = np.asarray(tgt_points, dtype=np.float32)
    weights = np.asarray(weights, dtype=np.float32)

    nc = _get_module()
    in_maps = _make_in_maps(src_points, tgt_points)
    res = run_bass_kernel_spmd(nc, in_maps, list(range(NCORES)))
    return _host_reduce(res.results, weights)
